# revision 2
# baseline (speedup 1.0000x reference)
"""Trainium2 Bass kernel: causal self-attention (B=2, T=2048, C=1024, H=16, Dh=64).

Sharding: 8 cores = 2 (batch) x 4 (head groups of 4 heads).  Each core gets
x[b] plus the W_qkv rows / W_proj columns for its heads, computes the full
attention + a partial output projection for its batch, and the host sums the
4 partials per batch (tensor-parallel unshard).

All matmuls run in bf16 with f32 PSUM accumulation.  x is passed transposed
(xT = x[b].T) so that:
  qT, kT = Wq @ xT, Wk @ xT     (head dim on partitions)  -- no transposes
  v      = xT.T @ WvT           (natural [T, d] layout)
  S^T    = kT_h(tile).T @ qT_h  ([k, q] layout, 128x512 blocks)
  exp on ScalarE (logits are bounded, no max pass needed); causal masking by
  computing only the live columns of each block (diagonal blocks truncate
  their dead leading columns in the S matmul, the exp, and the y matmul) plus
  one multiplicative [128,128] triangle mask on the diagonal subtile; row
  sums via a ones column appended to V (so P@[V|1] accumulates y^T and the
  softmax denominators in one PSUM tile); the 1/sum normalization applied on
  eviction using a PE-broadcast reciprocal row.
  out_partial = y^T.T @ WpT   (f32, DMA'd out).

ScalarE runs nothing but Exp during the attention phase -- any other
activation function (even Copy) can trigger a ~2.7us ACT table reload.
"""
import sys
import types

import numpy as np
import ml_dtypes

_BF16 = ml_dtypes.bfloat16


def _install_ntff_hook():
    """Provide antenv.axon_hooks so run_bass_kernel_spmd(trace=True) works."""
    if "antenv.axon_hooks" in sys.modules:
        return
    mod = types.ModuleType("antenv.axon_hooks")
    mod._hook = None

    def set_axon_ntff_profile_hook(h):
        mod._hook = h

    def get_axon_ntff_profile_hook():
        return mod._hook

    mod.set_axon_ntff_profile_hook = set_axon_ntff_profile_hook
    mod.get_axon_ntff_profile_hook = get_axon_ntff_profile_hook
    sys.modules["antenv.axon_hooks"] = mod
    try:
        import antenv

        antenv.axon_hooks = mod
    except Exception:
        pass
    try:
        from trn_agent_boot.trn_boot import _ntff_profile_via_ctypes

        mod.set_axon_ntff_profile_hook(
            _ntff_profile_via_ctypes("/opt/axon/libaxon_pjrt.so")
        )
    except Exception:
        pass


_install_ntff_hook()

import concourse.bacc as bacc
import concourse.mybir as mybir
from concourse import bass_utils
from concourse.tile import TileContext

# no network bucket in this container; keep artifacts local
bass_utils.upload_artifacts = lambda tmpdir: tmpdir

BF16 = mybir.dt.bfloat16
F32 = mybir.dt.float32

B, T, C = 2, 2048, 1024
H, D = 16, 64
HL = 4            # heads per core
OL = HL * D       # 256 local qkv output dim
P = 128
KC = C // P       # 8 contraction chunks
NQT = T // P      # 16 q/k 128-tiles
NQC = T // 512    # 4 q 512-chunks
VA = D + 1        # v columns per head incl. ones column (65)

_nc_cache = None


def _build_nc():
    nc = bacc.Bacc("TRN2", target_bir_lowering=False, debug=False, num_devices=8)

    xT = nc.declare_dram_parameter("xT", [C, T], BF16, isOutput=False)
    wqT = nc.declare_dram_parameter("wqT", [C, OL], BF16, isOutput=False)
    wkT = nc.declare_dram_parameter("wkT", [C, OL], BF16, isOutput=False)
    wvT = nc.declare_dram_parameter("wvT", [C, OL], BF16, isOutput=False)
    wpT = nc.declare_dram_parameter("wpT", [OL, C], BF16, isOutput=False)
    mk = nc.declare_dram_parameter("mask_tri", [P, P], BF16, isOutput=False)
    out = nc.declare_dram_parameter("out", [T, C], F32, isOutput=True)

    Exp = mybir.ActivationFunctionType.Exp

    with TileContext(nc) as tc:
        with tc.tile_pool(name="const", bufs=1) as const, \
             tc.tile_pool(name="misc", bufs=2) as misc, \
             tc.tile_pool(name="att", bufs=4) as att, \
             tc.tile_pool(name="outp", bufs=3) as outp:
            xT_sb = const.tile([P, KC * T], BF16, name="xT_sb")
            wq_sb = const.tile([P, KC * OL], BF16, name="wq_sb")
            wk_sb = const.tile([P, KC * OL], BF16, name="wk_sb")
            wv_sb = const.tile([P, KC * OL], BF16, name="wv_sb")
            wp_sb = const.tile([P, 2 * C], BF16, name="wp_sb")
            mk_sb = const.tile([P, P], BF16, name="mk_sb")
            ones_sb = const.tile([1, P], F32, name="ones_sb")
            qT_sb = const.tile([P, 2 * T], BF16, name="qT_sb")
            kT_sb = const.tile([P, 2 * T], BF16, name="kT_sb")
            va_sb = const.tile([P, NQT * HL * VA], BF16, name="va_sb")
            yT_sb = const.tile([P, 2 * T], BF16, name="yT_sb")

            # ---- input DMAs (xT streamed by 512-col pieces, tch-major) ----
            for w_sb, w_dram in ((wq_sb, wqT), (wk_sb, wkT), (wv_sb, wvT)):
                nc.sync.dma_start(
                    out=w_sb[:, :].rearrange("p (n o) -> p n o", n=KC),
                    in_=w_dram[:, :].rearrange("(n p) o -> p n o", p=P),
                )
            for tch in range(NQC):
                for n in range(KC):
                    nc.sync.dma_start(
                        out=xT_sb[:, n * T + tch * 512: n * T + tch * 512 + 512],
                        in_=xT[n * P:(n + 1) * P, tch * 512:(tch + 1) * 512],
                    )
            nc.sync.dma_start(
                out=wp_sb[:, :].rearrange("p (n o) -> p n o", n=2),
                in_=wpT[:, :].rearrange("(n p) o -> p n o", p=P),
            )
            nc.sync.dma_start(out=mk_sb[:, :], in_=mk[:, :])
            nc.vector.memset(ones_sb[:, :], 1.0)
            va_view = va_sb[:, :].rearrange("p (t h e) -> p t h e", t=NQT, h=HL)
            nc.vector.memset(va_view[:, :, :, D:VA], 1.0)

            # ---- phase 1: QKV projections ----
            # emission order brings heads 0/1 (oc=0) + early v tiles up first
            # so attention can overlap the rest of the phase.
            with tc.tile_pool(name="qkv_ps", bufs=4, space="PSUM") as qkv_pool:
                def qk_tile(w_sb, dst_sb, oc, tch):
                    ps = qkv_pool.tile([P, 512], F32, name="qkps", tag="qkvps")
                    for kc in range(KC):
                        nc.tensor.matmul(
                            ps[:, :],
                            w_sb[:, kc * OL + oc * P: kc * OL + oc * P + P],
                            xT_sb[:, kc * T + tch * 512: kc * T + tch * 512 + 512],
                            start=(kc == 0),
                            stop=(kc == KC - 1),
                        )
                    nc.scalar.copy(
                        dst_sb[:, oc * T + tch * 512: oc * T + tch * 512 + 512],
                        ps[:, :],
                    )

                def v_tile(tt):
                    ps = qkv_pool.tile([P, 512], F32, name="vps", tag="qkvps")
                    for kc in range(KC):
                        nc.tensor.matmul(
                            ps[:, 0:OL],
                            xT_sb[:, kc * T + tt * P: kc * T + tt * P + P],
                            wv_sb[:, kc * OL:(kc + 1) * OL],
                            start=(kc == 0),
                            stop=(kc == KC - 1),
                        )
                    nc.scalar.copy(
                        va_view[:, tt, :, 0:D],
                        ps[:, 0:OL].rearrange("p (h d) -> p h d", h=HL),
                    )

                for tch in range(NQC):
                    qk_tile(wq_sb, qT_sb, 0, tch)
                    qk_tile(wk_sb, kT_sb, 0, tch)
                    for tt in range(4 * tch, 4 * tch + 4):
                        v_tile(tt)
                for tch in range(NQC):
                    qk_tile(wq_sb, qT_sb, 1, tch)
                    qk_tile(wk_sb, kT_sb, 1, tch)

            # ---- phase 2: attention (+ interleaved projection) ----
            with tc.tile_pool(name="s_ps", bufs=3, space="PSUM") as s_pool, \
                 tc.tile_pool(name="y_ps", bufs=2, space="PSUM") as y_pool, \
                 tc.tile_pool(name="bc_ps", bufs=1, space="PSUM") as bc_pool, \
                 tc.tile_pool(name="pr_ps", bufs=2, space="PSUM") as pr_pool:
                for j4 in range(NQC):
                    q0 = j4 * 512
                    for h in range(HL):
                        po = 64 * (h % 2)
                        ch = h // 2
                        y_ps = y_pool.tile([P, 512], F32, name="yps", tag="yps")
                        nk = 4 * (j4 + 1)
                        for i in range(nk):
                            # diagonal blocks: leading 128*m0 cols are fully
                            # masked -- skip them in S, exp and y entirely.
                            m0 = max(0, i - 4 * j4)
                            c0 = 128 * m0
                            s_ps = s_pool.tile([P, 512], F32, name="sps", tag="sps")
                            nc.tensor.matmul(
                                s_ps[:, c0:512],
                                kT_sb[po:po + D, ch * T + i * P: ch * T + i * P + P],
                                qT_sb[po:po + D, ch * T + q0 + c0: ch * T + q0 + 512],
                                start=True,
                                stop=True,
                            )
                            p_t = att.tile([P, 512], BF16, name="pt", tag="pt")
                            nc.scalar.activation(
                                p_t[:, c0:512], s_ps[:, c0:512], Exp, scale=0.125
                            )
                            if i >= 4 * j4:
                                nc.vector.tensor_mul(
                                    p_t[:, c0:c0 + P], p_t[:, c0:c0 + P], mk_sb[:, :]
                                )
                            nc.tensor.matmul(
                                y_ps[0:VA, c0:512],
                                va_sb[:, (i * HL + h) * VA:(i * HL + h) * VA + VA],
                                p_t[:, c0:512],
                                start=(i == 0),
                                stop=(i == nk - 1),
                            )
                        # evict y to SBUF (frees the PSUM bank), then normalize
                        y_sb = misc.tile([P, 512], F32, name="ysb", tag="ysb")
                        nc.vector.tensor_copy(y_sb[0:VA, :], y_ps[0:VA, :])
                        rc = misc.tile([1, 512], F32, name="rc", tag="rc")
                        nc.vector.reciprocal(rc[:, :], y_sb[D:VA, :])
                        bc_ps = bc_pool.tile([P, 512], F32, name="bcps", tag="bcps")
                        nc.tensor.matmul(
                            bc_ps[:, :], ones_sb[0:1, :], rc[:, :],
                            start=True, stop=True,
                        )
                        nc.vector.tensor_mul(
                            yT_sb[po:po + D, ch * T + q0: ch * T + q0 + 512],
                            y_sb[0:D, :],
                            bc_ps[0:D, :],
                        )
                    # projection over this finished q-chunk
                    for tt in range(4):
                        trow = (j4 * 4 + tt) * P
                        for ocn in range(2):
                            pr_ps = pr_pool.tile([P, 512], F32, name="prps", tag="prps")
                            for cc in range(2):
                                nc.tensor.matmul(
                                    pr_ps[:, :],
                                    yT_sb[:, cc * T + trow: cc * T + trow + P],
                                    wp_sb[:, cc * C + ocn * 512: cc * C + ocn * 512 + 512],
                                    start=(cc == 0),
                                    stop=(cc == 1),
                                )
                            o_sb = outp.tile([P, 512], F32, name="osb", tag="osb")
                            nc.vector.tensor_copy(o_sb[:, :], pr_ps[:, :])
                            nc.sync.dma_start(
                                out=out[trow:trow + P, ocn * 512:(ocn + 1) * 512],
                                in_=o_sb[:, :],
                            )
    nc.compile()
    return nc


def _get_nc():
    global _nc_cache
    if _nc_cache is None:
        _nc_cache = _build_nc()
    return _nc_cache


def _prepare_in_maps(x, W_qkv, W_proj):
    x = np.asarray(x, np.float32)
    W_qkv = np.asarray(W_qkv, np.float32)
    W_proj = np.asarray(W_proj, np.float32)
    # [r, j] = 1 where j >= r (upper triangle incl diag, in S^T [k, q] layout)
    tri = (np.arange(P)[None, :] >= np.arange(P)[:, None]).astype(np.float32)
    tri = tri.astype(_BF16)
    in_maps = []
    for c in range(8):
        b, g = c // 4, c % 4
        r0 = OL * g
        in_maps.append({
            "xT": np.ascontiguousarray(x[b].T).astype(_BF16),
            "wqT": np.ascontiguousarray(W_qkv[r0:r0 + OL, :].T).astype(_BF16),
            "wkT": np.ascontiguousarray(W_qkv[C + r0:C + r0 + OL, :].T).astype(_BF16),
            "wvT": np.ascontiguousarray(W_qkv[2 * C + r0:2 * C + r0 + OL, :].T).astype(_BF16),
            "wpT": np.ascontiguousarray(W_proj[:, r0:r0 + OL].T).astype(_BF16),
            "mask_tri": tri,
        })
    return in_maps


def _combine(results):
    out = np.zeros((B, T, C), np.float32)
    for c in range(8):
        out[c // 4] += results[c]["out"]
    return out


def kernel(x, W_qkv, W_proj):
    nc = _get_nc()
    in_maps = _prepare_in_maps(x, W_qkv, W_proj)
    res = bass_utils.run_bass_kernel_spmd(nc, in_maps, core_ids=list(range(8)))
    return _combine(res.results)


def kernel_traced(x, W_qkv, W_proj, trace_cores=None):
    """Like kernel() but returns (out, exec_time_ns) using an NTFF profile."""
    nc = _get_nc()
    in_maps = _prepare_in_maps(x, W_qkv, W_proj)
    res = bass_utils.run_bass_kernel_spmd(
        nc, in_maps, core_ids=list(range(8)), trace=True, trace_cores=trace_cores
    )
    return _combine(res.results), res.exec_time_ns


# revision 3
# speedup vs baseline: 1.0080x; 1.0080x over previous
"""Trainium2 Bass kernel: causal self-attention (B=2, T=2048, C=1024, H=16, Dh=64).

Sharding: 8 cores = 2 (batch) x 4 (head groups of 4 heads).  Each core gets
x[b] plus the W_qkv rows / W_proj columns for its heads, computes the full
attention + a partial output projection for its batch, and the host sums the
4 partials per batch (tensor-parallel unshard).

All matmuls run in bf16 with f32 PSUM accumulation.  x is passed transposed
(xT = x[b].T) so that:
  qT, kT = Wq @ xT, Wk @ xT     (head dim on partitions)  -- no transposes
  v      = xT.T @ WvT           (natural [T, d] layout)
  S^T    = kT_h(tile).T @ qT_h  ([k, q] layout, 128x512 blocks)
  exp on ScalarE (logits are bounded, no max pass needed); causal masking by
  computing only the live columns of each block (diagonal blocks truncate
  their dead leading columns in the S matmul, the exp, and the y matmul) plus
  one multiplicative [128,128] triangle mask on the diagonal subtile; row
  sums via a ones column appended to V (so P@[V|1] accumulates y^T and the
  softmax denominators in one PSUM tile); the 1/sum normalization applied on
  eviction using a PE-broadcast reciprocal row.
  out_partial = y^T.T @ WpT   (f32, DMA'd out).

ScalarE runs nothing but Exp during the attention phase -- any other
activation function (even Copy) can trigger a ~2.7us ACT table reload.
"""
import sys
import types

import numpy as np
import ml_dtypes

_BF16 = ml_dtypes.bfloat16


def _install_ntff_hook():
    """Provide antenv.axon_hooks so run_bass_kernel_spmd(trace=True) works."""
    if "antenv.axon_hooks" in sys.modules:
        return
    mod = types.ModuleType("antenv.axon_hooks")
    mod._hook = None

    def set_axon_ntff_profile_hook(h):
        mod._hook = h

    def get_axon_ntff_profile_hook():
        return mod._hook

    mod.set_axon_ntff_profile_hook = set_axon_ntff_profile_hook
    mod.get_axon_ntff_profile_hook = get_axon_ntff_profile_hook
    sys.modules["antenv.axon_hooks"] = mod
    try:
        import antenv

        antenv.axon_hooks = mod
    except Exception:
        pass
    try:
        from trn_agent_boot.trn_boot import _ntff_profile_via_ctypes

        mod.set_axon_ntff_profile_hook(
            _ntff_profile_via_ctypes("/opt/axon/libaxon_pjrt.so")
        )
    except Exception:
        pass


_install_ntff_hook()

import concourse.bacc as bacc
import concourse.mybir as mybir
from concourse import bass_utils
from concourse.tile import TileContext

# no network bucket in this container; keep artifacts local
bass_utils.upload_artifacts = lambda tmpdir: tmpdir

BF16 = mybir.dt.bfloat16
F32 = mybir.dt.float32

B, T, C = 2, 2048, 1024
H, D = 16, 64
HL = 4            # heads per core
OL = HL * D       # 256 local qkv output dim
P = 128
KC = C // P       # 8 contraction chunks
NQT = T // P      # 16 q/k 128-tiles
NQC = T // 512    # 4 q 512-chunks
VA = D + 1        # v columns per head incl. ones column (65)

_nc_cache = None


def _build_nc():
    nc = bacc.Bacc("TRN2", target_bir_lowering=False, debug=False, num_devices=8)

    xT = nc.declare_dram_parameter("xT", [C, T], BF16, isOutput=False)
    wqT = nc.declare_dram_parameter("wqT", [C, OL], BF16, isOutput=False)
    wkT = nc.declare_dram_parameter("wkT", [C, OL], BF16, isOutput=False)
    wvT = nc.declare_dram_parameter("wvT", [C, OL], BF16, isOutput=False)
    wpT = nc.declare_dram_parameter("wpT", [OL, C], BF16, isOutput=False)
    mk = nc.declare_dram_parameter("mask_tri", [P, P], BF16, isOutput=False)
    out = nc.declare_dram_parameter("out", [T, C], F32, isOutput=True)

    Exp = mybir.ActivationFunctionType.Exp

    with TileContext(nc) as tc:
        with tc.tile_pool(name="const", bufs=1) as const, \
             tc.tile_pool(name="misc", bufs=2) as misc, \
             tc.tile_pool(name="att", bufs=4) as att, \
             tc.tile_pool(name="outp", bufs=3) as outp:
            xT_sb = const.tile([P, KC * T], BF16, name="xT_sb")
            wq_sb = const.tile([P, KC * OL], BF16, name="wq_sb")
            wk_sb = const.tile([P, KC * OL], BF16, name="wk_sb")
            wv_sb = const.tile([P, KC * OL], BF16, name="wv_sb")
            wp_sb = const.tile([P, 2 * C], BF16, name="wp_sb")
            mk_sb = const.tile([P, P], BF16, name="mk_sb")
            ones_sb = const.tile([1, P], F32, name="ones_sb")
            qT_sb = const.tile([P, 2 * T], BF16, name="qT_sb")
            kT_sb = const.tile([P, 2 * T], BF16, name="kT_sb")
            va_sb = const.tile([P, NQT * HL * VA], BF16, name="va_sb")
            yT_sb = const.tile([P, 2 * T], BF16, name="yT_sb")

            # ---- input DMAs (xT streamed by 512-col pieces, tch-major) ----
            for w_sb, w_dram in ((wq_sb, wqT), (wk_sb, wkT), (wv_sb, wvT)):
                nc.sync.dma_start(
                    out=w_sb[:, :].rearrange("p (n o) -> p n o", n=KC),
                    in_=w_dram[:, :].rearrange("(n p) o -> p n o", p=P),
                )
            for tch in range(NQC):
                for n in range(KC):
                    nc.sync.dma_start(
                        out=xT_sb[:, n * T + tch * 512: n * T + tch * 512 + 512],
                        in_=xT[n * P:(n + 1) * P, tch * 512:(tch + 1) * 512],
                    )
            nc.sync.dma_start(
                out=wp_sb[:, :].rearrange("p (n o) -> p n o", n=2),
                in_=wpT[:, :].rearrange("(n p) o -> p n o", p=P),
            )
            nc.sync.dma_start(out=mk_sb[:, :], in_=mk[:, :])
            nc.vector.memset(ones_sb[:, :], 1.0)
            va_view = va_sb[:, :].rearrange("p (t h e) -> p t h e", t=NQT, h=HL)
            nc.vector.memset(va_view[:, :, :, D:VA], 1.0)

            # ---- phase 1: QKV projections ----
            # emission order brings heads 0/1 (oc=0) + early v tiles up first
            # so attention can overlap the rest of the phase.
            with tc.tile_pool(name="qkv_ps", bufs=4, space="PSUM") as qkv_pool:
                def qk_tile(w_sb, dst_sb, oc, tch):
                    ps = qkv_pool.tile([P, 512], F32, name="qkps", tag="qkvps")
                    for kc in range(KC):
                        nc.tensor.matmul(
                            ps[:, :],
                            w_sb[:, kc * OL + oc * P: kc * OL + oc * P + P],
                            xT_sb[:, kc * T + tch * 512: kc * T + tch * 512 + 512],
                            start=(kc == 0),
                            stop=(kc == KC - 1),
                        )
                    nc.scalar.copy(
                        dst_sb[:, oc * T + tch * 512: oc * T + tch * 512 + 512],
                        ps[:, :],
                    )

                def v_tile(tt):
                    ps = qkv_pool.tile([P, 512], F32, name="vps", tag="qkvps")
                    for kc in range(KC):
                        nc.tensor.matmul(
                            ps[:, 0:OL],
                            xT_sb[:, kc * T + tt * P: kc * T + tt * P + P],
                            wv_sb[:, kc * OL:(kc + 1) * OL],
                            start=(kc == 0),
                            stop=(kc == KC - 1),
                        )
                    nc.scalar.copy(
                        va_view[:, tt, :, 0:D],
                        ps[:, 0:OL].rearrange("p (h d) -> p h d", h=HL),
                    )

                for tch in range(NQC):
                    qk_tile(wq_sb, qT_sb, 0, tch)
                    qk_tile(wk_sb, kT_sb, 0, tch)
                    for tt in range(4 * tch, 4 * tch + 4):
                        v_tile(tt)
                for tch in range(NQC):
                    qk_tile(wq_sb, qT_sb, 1, tch)
                    qk_tile(wk_sb, kT_sb, 1, tch)

            # ---- phase 2: attention (+ interleaved projection) ----
            with tc.tile_pool(name="s_ps", bufs=3, space="PSUM") as s_pool, \
                 tc.tile_pool(name="y_ps", bufs=2, space="PSUM") as y_pool, \
                 tc.tile_pool(name="bc_ps", bufs=1, space="PSUM") as bc_pool, \
                 tc.tile_pool(name="pr_ps", bufs=2, space="PSUM") as pr_pool:
                for j4 in range(NQC):
                    q0 = j4 * 512
                    for h in range(HL):
                        po = 64 * (h % 2)
                        ch = h // 2
                        y_ps = y_pool.tile([P, 512], F32, name="yps", tag="yps")
                        nk = 4 * (j4 + 1)
                        for i in range(nk):
                            # diagonal blocks: leading 128*m0 cols are fully
                            # masked -- skip them in S, exp and y entirely.
                            m0 = max(0, i - 4 * j4)
                            c0 = 128 * m0
                            s_ps = s_pool.tile([P, 512], F32, name="sps", tag="sps")
                            nc.tensor.matmul(
                                s_ps[:, c0:512],
                                kT_sb[po:po + D, ch * T + i * P: ch * T + i * P + P],
                                qT_sb[po:po + D, ch * T + q0 + c0: ch * T + q0 + 512],
                                start=True,
                                stop=True,
                            )
                            p_t = att.tile([P, 512], BF16, name="pt", tag="pt")
                            nc.scalar.activation(
                                p_t[:, c0:512], s_ps[:, c0:512], Exp, scale=0.125
                            )
                            if i >= 4 * j4:
                                # gpsimd, not DVE: DVE is in-order and busy
                                # with the per-head eviction chain; a DVE
                                # mask here stalls the exp->y pipeline.
                                nc.gpsimd.tensor_mul(
                                    p_t[:, c0:c0 + P], p_t[:, c0:c0 + P], mk_sb[:, :]
                                )
                            nc.tensor.matmul(
                                y_ps[0:VA, c0:512],
                                va_sb[:, (i * HL + h) * VA:(i * HL + h) * VA + VA],
                                p_t[:, c0:512],
                                start=(i == 0),
                                stop=(i == nk - 1),
                            )
                        # evict y to SBUF (frees the PSUM bank), then normalize
                        y_sb = misc.tile([P, 512], F32, name="ysb", tag="ysb")
                        nc.vector.tensor_copy(y_sb[0:VA, :], y_ps[0:VA, :])
                        rc = misc.tile([1, 512], F32, name="rc", tag="rc")
                        nc.vector.reciprocal(rc[:, :], y_sb[D:VA, :])
                        bc_ps = bc_pool.tile([P, 512], F32, name="bcps", tag="bcps")
                        nc.tensor.matmul(
                            bc_ps[:, :], ones_sb[0:1, :], rc[:, :],
                            start=True, stop=True,
                        )
                        nc.vector.tensor_mul(
                            yT_sb[po:po + D, ch * T + q0: ch * T + q0 + 512],
                            y_sb[0:D, :],
                            bc_ps[0:D, :],
                        )
                    # projection over this finished q-chunk
                    for tt in range(4):
                        trow = (j4 * 4 + tt) * P
                        for ocn in range(2):
                            pr_ps = pr_pool.tile([P, 512], F32, name="prps", tag="prps")
                            for cc in range(2):
                                nc.tensor.matmul(
                                    pr_ps[:, :],
                                    yT_sb[:, cc * T + trow: cc * T + trow + P],
                                    wp_sb[:, cc * C + ocn * 512: cc * C + ocn * 512 + 512],
                                    start=(cc == 0),
                                    stop=(cc == 1),
                                )
                            o_sb = outp.tile([P, 512], F32, name="osb", tag="osb")
                            nc.vector.tensor_copy(o_sb[:, :], pr_ps[:, :])
                            nc.sync.dma_start(
                                out=out[trow:trow + P, ocn * 512:(ocn + 1) * 512],
                                in_=o_sb[:, :],
                            )
    nc.compile()
    return nc


def _get_nc():
    global _nc_cache
    if _nc_cache is None:
        _nc_cache = _build_nc()
    return _nc_cache


def _prepare_in_maps(x, W_qkv, W_proj):
    x = np.asarray(x, np.float32)
    W_qkv = np.asarray(W_qkv, np.float32)
    W_proj = np.asarray(W_proj, np.float32)
    # [r, j] = 1 where j >= r (upper triangle incl diag, in S^T [k, q] layout)
    tri = (np.arange(P)[None, :] >= np.arange(P)[:, None]).astype(np.float32)
    tri = tri.astype(_BF16)
    in_maps = []
    for c in range(8):
        b, g = c // 4, c % 4
        r0 = OL * g
        in_maps.append({
            "xT": np.ascontiguousarray(x[b].T).astype(_BF16),
            "wqT": np.ascontiguousarray(W_qkv[r0:r0 + OL, :].T).astype(_BF16),
            "wkT": np.ascontiguousarray(W_qkv[C + r0:C + r0 + OL, :].T).astype(_BF16),
            "wvT": np.ascontiguousarray(W_qkv[2 * C + r0:2 * C + r0 + OL, :].T).astype(_BF16),
            "wpT": np.ascontiguousarray(W_proj[:, r0:r0 + OL].T).astype(_BF16),
            "mask_tri": tri,
        })
    return in_maps


def _combine(results):
    out = np.zeros((B, T, C), np.float32)
    for c in range(8):
        out[c // 4] += results[c]["out"]
    return out


def kernel(x, W_qkv, W_proj):
    nc = _get_nc()
    in_maps = _prepare_in_maps(x, W_qkv, W_proj)
    res = bass_utils.run_bass_kernel_spmd(nc, in_maps, core_ids=list(range(8)))
    return _combine(res.results)


def kernel_traced(x, W_qkv, W_proj, trace_cores=None):
    """Like kernel() but returns (out, exec_time_ns) using an NTFF profile."""
    nc = _get_nc()
    in_maps = _prepare_in_maps(x, W_qkv, W_proj)
    res = bass_utils.run_bass_kernel_spmd(
        nc, in_maps, core_ids=list(range(8)), trace=True, trace_cores=trace_cores
    )
    return _combine(res.results), res.exec_time_ns
